# revision 33
# baseline (speedup 1.0000x reference)
"""ChebConv (K=3) forward as a distributed Bass/Tile kernel on 8 trn2 NeuronCores.

Sharding: vertices V are sharded across the 8 cores (rows of L and x).
  x0 = [x[0] | x[1]]            # [V, 128], feature col = b*64 + fin
  x1 = L @ x0                   # SpMM (COO, edge-parallel)
  x2' = L @ x1 - 0.5 x0         # = x2/2; the 2x is folded into W_k2
  out[b,v,:] = bias + sum_k xk[v, b*64:(b+1)*64] @ Wk'

Each core owns a row shard (V/8 rows padded to 98 blocks of 128). SpMM per
core and per 128-edge tile (bf16 data, PSUM f32 accumulate):
  - gpsimd.dma_gather fetches 256B source feature rows from a flat-indexed
    replicated table (int16 indices, 4 chunks), round-robin over 4 SWDGE
    queues.
  - The selector mask[e,j] = (lrow[e]==j) is built ON-CHIP by the (otherwise
    idle) Vector engine from a streamed lrow row (one broadcast-compare per
    run); the edge value is folded into the gathered rows (gs = val * g,
    second DVE op). No selector matrix is ever streamed from HBM.
  - PE matmul mask^T @ gs (spmm1) or gs^T @ mask (spmm2) performs the scaled
    segmented sum into a per-block PSUM accumulator.
Edge metadata (idx / lrow / val) is loaded once into SBUF and shared by both
SpMMs. x0^T blocks are bulk-loaded resident; x1^T blocks are produced by a PE
transpose in phase 1 and kept resident, so phase 2 reloads nothing per-block.

The vertex flat index space is PIECE-major: blocks [0:28), [28:56), [56:84),
[84:98) of every core form 4 global pieces; the x1 AllGather is split into 4
piece collectives so the first 3 overlap with the tail of SpMM1. Gather
chunks coincide with pieces (each <= 32768 rows, int16-addressable).

The final channel mix is fused into SpMM2's block epilogue using
block-diagonal weights (both batches in one matmul chain) plus a rank-1 bias
matmul; -0.5 x0 enters via a DVE (x0^T * -0.5 + psum) fused copy.
"""

import sys

sys.path.insert(0, "/opt/trn_rl_repo")

import numpy as np
import ml_dtypes

import concourse.bass as bass
import concourse.bacc as bacc
import concourse.mybir as mybir
import concourse.tile as tile
from concourse import bass_utils
from concourse.alu_op_type import AluOpType

P = 128
F32 = mybir.dt.float32
BF16 = mybir.dt.bfloat16
I16 = mybir.dt.int16
NPBF16 = ml_dtypes.bfloat16
NQ = 4  # SWDGE queues (parallel Q7 descriptor generation)
SB = 7  # blocks per super-block (PSUM ring = SB accumulators)
RT = 12  # tiles per gather sub-run (small gathers overlap desc-gen better)
PIECE_BLOCKS = (28, 28, 28, 14)  # 4 pieces x (blocks/core); sbs of 7 align


def _cdiv(a, b):
    return -(-a // b)


# ---------------------------------------------------------------------------
# Host-side: uniform (cross-core) edge structure + per-core content arrays
# ---------------------------------------------------------------------------


def _pack_piece(dp, nb, caps, rounds=4, max_iter=60000):
    """Assign len(dp) rows (4-dim chunk-degree vectors) to nb blocks of 128
    rows each, fitting per-(block,chunk) counts under caps via swap search."""
    n = len(dp)
    order = np.argsort(-dp.sum(1), kind="stable")
    seq = np.empty(n, np.int64)
    blkseq = []
    fwd = True
    while len(blkseq) < n:
        blkseq.extend(range(nb) if fwd else range(nb - 1, -1, -1))
        fwd = not fwd
    seq[order] = np.array(blkseq[:n])
    assign = seq
    cnt = np.zeros((nb, 4), np.int64)
    np.add.at(cnt, (assign,), dp)
    it = 0
    for _ in range(rounds):
        blocked = np.zeros((nb, 4), bool)
        progress = False
        while it < max_iter:
            it += 1
            over = np.where(blocked, 0, cnt - caps)
            if over.max() <= 0:
                break
            b, g = np.unravel_index(np.argmax(over), over.shape)
            rows_b = np.nonzero(assign == b)[0]
            cand_i = rows_b[np.argsort(-dp[rows_b, g], kind="stable")[:4]]
            swapped = False
            for i in cand_i:
                di = dp[i]
                if di[g] == 0:
                    break
                f1 = np.all(cnt[assign] + di - dp <= caps[assign], axis=1)
                f1 &= assign != b
                after_b = cnt[b][None] - di[None] + dp
                f2 = np.all(
                    after_b <= np.maximum(caps[b][None], cnt[b][None]), axis=1
                )
                gain = di[g] - dp[:, g]
                ok = f1 & f2 & (gain > 0)
                if ok.any():
                    jj = np.nonzero(ok)[0]
                    j = jj[np.argmax(gain[jj])]
                    bj = assign[j]
                    assign[i], assign[j] = bj, b
                    cnt[b] += dp[j] - di
                    cnt[bj] += di - dp[j]
                    blocked[b, :] = False
                    blocked[bj, :] = False
                    swapped = True
                    progress = True
                    break
            if not swapped:
                blocked[b, g] = True
        if (cnt - caps).max() <= 0 or not progress:
            break
    return assign


class EdgeStructure:
    def __init__(self, V, ncores, rows, cols):
        assert V % ncores == 0
        self.V, self.ncores = V, ncores
        self.vsh = V // ncores
        self.nblk = _cdiv(self.vsh, P)
        self.vpad = self.nblk * P
        self.vtot = self.vpad * ncores
        assert sum(PIECE_BLOCKS) == self.nblk and self.nblk % SB == 0

        # piece-major flat layout: piece g holds ncores * PIECE_BLOCKS[g]
        # consecutive blocks (core-major within the piece)
        pb = np.asarray(PIECE_BLOCKS, np.int64)
        self.piece_start = np.concatenate(([0], np.cumsum(pb)))[:-1]
        self.piece_base = np.concatenate(([0], np.cumsum(pb * ncores * P)))[:-1]
        self.nchunks = len(pb)
        self.chunk_bounds = [
            (int(self.piece_base[g]), int(self.piece_base[g] + ncores * pb[g] * P))
            for g in range(self.nchunks)
        ]
        assert all(b - a <= 32768 for a, b in self.chunk_bounds)
        self.piece_of_block = np.searchsorted(
            self.piece_start, np.arange(self.nblk), side="right"
        ) - 1

        rows = np.asarray(rows, np.int64)
        cols = np.asarray(cols, np.int64)
        r_core = rows // self.vsh
        r_loc = rows - r_core * self.vsh
        c_of = cols // self.vsh
        l_of = cols - c_of * self.vsh

        # ---- row->block packing (within pieces) to minimize tile padding --
        colchunk0 = self.piece_of_block[l_of // P]
        d = np.zeros((ncores, self.vsh, 4), np.int32)
        np.add.at(d, (r_core, r_loc, colchunk0), 1)
        caps_all = np.zeros((self.nblk, 4), np.int64)
        for p in range(self.nchunks):
            b0, b1 = int(self.piece_start[p]), int(self.piece_start[p] + pb[p])
            nb = b1 - b0
            r0, r1 = b0 * P, min(b1 * P, self.vsh)
            tot = d[:, r0:r1].sum(axis=1)
            for g in range(self.nchunks):
                npg = -(-int(tot[:, g].max()) // P) + 1
                base, extra = npg // nb, npg % nb
                caps_all[b0:b1, g] = base * P
                caps_all[b0 : b0 + extra, g] += P
        self.perms = np.full((ncores, self.vpad), -1, np.int64)
        for c in range(ncores):
            for p in range(self.nchunks):
                b0 = int(self.piece_start[p])
                b1 = b0 + int(pb[p])
                nb = b1 - b0
                r0, r1 = b0 * P, min(b1 * P, self.vsh)
                n_real = r1 - r0
                n_tot = nb * P
                dp = np.zeros((n_tot, 4), np.int64)
                dp[:n_real] = d[c, r0:r1]
                assign = _pack_piece(dp, nb, caps_all[b0:b1])
                used = np.zeros(nb, np.int64)
                for idx in range(n_tot):
                    b = assign[idx]
                    self.perms[c, (b0 + b) * P + used[b]] = (
                        (r0 + idx) if idx < n_real else -1
                    )
                    used[b] += 1
        inv = np.full((ncores, self.vsh), -1, np.int64)
        for c in range(ncores):
            m = self.perms[c] >= 0
            inv[c, self.perms[c][m]] = np.nonzero(m)[0]

        # new (post-permutation) local positions for rows and columns
        r_loc = inv[r_core, r_loc]
        nl_col = inv[c_of, l_of]
        blk_col = nl_col // P
        g_col = self.piece_of_block[blk_col]
        flat = (
            self.piece_base[g_col]
            + c_of * pb[g_col] * P
            + (blk_col - self.piece_start[g_col]) * P
            + (nl_col - blk_col * P)
        )
        chunk = g_col
        blk = r_loc // P

        nchunks = self.nchunks
        # slot order: for sb: for chunk: for block in sb
        sb_arr = blk // SB
        bi_arr = blk % SB
        bh_arr = np.minimum(SB, self.nblk - sb_arr * SB)
        sid = sb_arr * SB * nchunks + chunk * bh_arr + bi_arr

        self.nsb = _cdiv(self.nblk, SB)
        order = []
        for sb in range(self.nsb):
            b0 = sb * SB
            bh = min(SB, self.nblk - b0)
            for ch in range(nchunks):
                for bi in range(bh):
                    order.append((b0 + bi, ch))
        self.nslots = len(order)
        self.slot_block = np.array([b for b, _ in order], np.int64)
        self.slot_chunk = np.array([c for _, c in order], np.int64)

        counts = np.zeros((ncores, self.nslots), np.int64)
        np.add.at(counts, (r_core, sid), 1)
        T = _cdiv(np.max(counts, axis=0), P)

        # every block needs >=1 tile so its PSUM accumulator gets written
        blk_tiles = np.zeros(self.nblk, np.int64)
        np.add.at(blk_tiles, self.slot_block, T)
        for b in np.nonzero(blk_tiles == 0)[0]:
            sb, bi = b // SB, b % SB
            bh = min(SB, self.nblk - sb * SB)
            T[sb * SB * nchunks + 0 * bh + bi] = 1

        self.T = T
        self.slot_tile_base = np.concatenate(([0], np.cumsum(T)))[:-1]
        self.ntiles = int(np.sum(T))
        self.sid_of_edge = sid
        self.flat_of_edge = flat
        self.chunk_of_edge = chunk
        self.r_core_of_edge = r_core
        self.lrow_of_edge = (r_loc % P).astype(np.int64)

        # (sb, chunk) -> contiguous tile run
        self.runs = []  # per sb: list of (tile_start, ntiles, chunk)
        s = 0
        for sb in range(self.nsb):
            b0 = sb * SB
            bh = min(SB, self.nblk - b0)
            sb_runs = []
            for ch in range(nchunks):
                t0 = int(self.slot_tile_base[s])
                ntr = int(np.sum(T[s : s + bh]))
                if ntr > 0:
                    sb_runs.append((t0, ntr, ch))
                s += bh
            self.runs.append(sb_runs)
        self.max_run_tiles = max(
            nt for sb_runs in self.runs for _, nt, _ in sb_runs
        )

        tile_block = np.empty(self.ntiles, np.int64)
        for s in range(self.nslots):
            t0, ntr = self.slot_tile_base[s], T[s]
            tile_block[t0 : t0 + ntr] = self.slot_block[s]
        self.tile_block = tile_block
        self.tile_start = np.zeros(self.ntiles, bool)
        self.tile_stop = np.zeros(self.ntiles, bool)
        first, last = {}, {}
        for t in range(self.ntiles):
            b = int(tile_block[t])
            if b not in first:
                first[b] = t
            last[b] = t
        for t in first.values():
            self.tile_start[t] = True
        for t in last.values():
            self.tile_stop[t] = True

    def per_core_arrays(self, core, vals):
        """int16 gather indices (wrapped+replicated) and bf16 lrow/val rows."""
        sel = np.nonzero(self.r_core_of_edge == core)[0]
        sid = self.sid_of_edge[sel]
        o = np.argsort(sid, kind="stable")
        sel, sid = sel[o], sid[o]
        start = np.searchsorted(sid, np.arange(self.nslots))
        rank = np.arange(len(sid)) - start[sid]
        pos = self.slot_tile_base[sid] * P + rank
        n = self.ntiles * P
        idx = np.zeros(n, np.int16)
        idx[pos] = (
            self.flat_of_edge[sel] - self.piece_base[self.slot_chunk[sid]]
        ).astype(np.int16)
        idx_w = np.tile(np.ascontiguousarray(idx.reshape(-1, 16).T), (8, 1))
        lrow = np.zeros((P, self.ntiles), np.float32)
        val = np.zeros((P, self.ntiles), np.float32)
        lrow[pos % P, pos // P] = self.lrow_of_edge[sel]
        val[pos % P, pos // P] = vals[sel]
        return idx_w, lrow.astype(NPBF16), val.astype(NPBF16)

    def shard_permuted(self, x0_bf16, c):
        """Core c's [vpad, 128] shard in post-packing row order."""
        xs = np.zeros((self.vpad, P), NPBF16)
        m = self.perms[c] >= 0
        xs[m] = x0_bf16[c * self.vsh + self.perms[c][m]]
        return xs

    def x_to_flat(self, x0_bf16):
        """Scatter [V, 128] rows into the piece-major padded flat table."""
        xf = np.zeros((self.vtot, P), NPBF16)
        pb = np.asarray(PIECE_BLOCKS, np.int64)
        for c in range(self.ncores):
            xs = self.shard_permuted(x0_bf16, c)
            for g in range(self.nchunks):
                r0 = self.piece_start[g] * P
                r1 = r0 + pb[g] * P
                dst = self.piece_base[g] + c * pb[g] * P
                xf[dst : dst + (r1 - r0)] = xs[r0:r1]
        return xf


# ---------------------------------------------------------------------------
# Bass program (SPMD: one program, per-core data via in_maps)
# ---------------------------------------------------------------------------


def build_program(es: EdgeStructure):
    nblk, vpad, vtot, ncores = es.nblk, es.vpad, es.vtot, es.ncores
    nt, GW = es.ntiles, es.max_run_tiles

    nc = bacc.Bacc(
        "TRN2",
        target_bir_lowering=False,
        debug=False,
        num_devices=ncores,
        num_swdge_queues=NQ,
    )

    x0f = nc.dram_tensor("x0f", [vtot, P], BF16, kind="ExternalInput")
    x0t = nc.dram_tensor("x0t", [P, nblk * P], BF16, kind="ExternalInput")
    wbd = nc.dram_tensor("wbd", [3, P, P], BF16, kind="ExternalInput")
    biasbd = nc.dram_tensor("biasbd", [1, P], BF16, kind="ExternalInput")
    ident_d = nc.dram_tensor("ident", [P, P], BF16, kind="ExternalInput")
    ones_d = nc.dram_tensor("ones1", [1, P], BF16, kind="ExternalInput")
    iota_d = nc.dram_tensor("iota", [P, P], BF16, kind="ExternalInput")
    eidx = nc.dram_tensor("eidx", [P, nt * 8], I16, kind="ExternalInput")
    elrow = nc.dram_tensor("elrow", [P, nt], BF16, kind="ExternalInput")
    eval_ = nc.dram_tensor("eval", [P, nt], BF16, kind="ExternalInput")
    outp = nc.dram_tensor("outp", [vpad, P], F32, kind="ExternalOutput")

    x1my = nc.dram_tensor("x1my", [vpad, P], BF16)
    x1full = nc.dram_tensor("x1full", [vtot, P], BF16, addr_space="Shared")

    pb = PIECE_BLOCKS
    nsb_total = sum(pb) // SB
    piece_after_sb = {}  # sb index -> piece idx launched at its end
    acc = 0
    for g, n in enumerate(pb):
        acc += n
        # launch one sb late (except the last piece): by then the piece's
        # x1my stores have drained, so the collective's sem-wait doesn't
        # head-of-line-block gather desc-gen on the gpsimd queue
        sb_done = acc // SB - 1
        piece_after_sb[min(sb_done + 1, nsb_total - 1)] = g

    with tile.TileContext(nc) as tc:
        with (
            tc.tile_pool(name="const", bufs=1) as cpool,
            tc.tile_pool(name="gslab", bufs=8) as gpool0,
            tc.tile_pool(name="mvslab", bufs=8) as mvpool,
            tc.tile_pool(name="mslab", bufs=3) as mpool,
            tc.tile_pool(name="ostage", bufs=4) as opool,
            tc.tile_pool(name="acc", bufs=SB, space="PSUM") as apool,
            tc.tile_pool(name="paux", bufs=1, space="PSUM") as auxpool,
        ):
            # ---- resident data (loaded once, shared by both SpMMs) ----
            ident_s = cpool.tile([P, P], BF16, tag="ident")
            nc.scalar.dma_start(out=ident_s[:], in_=ident_d[:, :])
            ones_s = cpool.tile([1, P], BF16, tag="ones")
            nc.scalar.dma_start(out=ones_s[:], in_=ones_d[:, :])
            bias_s = cpool.tile([1, P], BF16, tag="bias")
            nc.scalar.dma_start(out=bias_s[:], in_=biasbd[:, :])
            iota_s = cpool.tile([P, P], BF16, tag="iota")
            nc.scalar.dma_start(out=iota_s[:], in_=iota_d[:, :])
            wbd_s = cpool.tile([P, 3 * P], BF16, tag="wbd")
            for k in range(3):
                nc.scalar.dma_start(
                    out=wbd_s[:, k * P : (k + 1) * P], in_=wbd[k, :, :]
                )
            lrow_s = cpool.tile([P, nt], BF16, tag="lrow")
            nc.scalar.dma_start(out=lrow_s[:], in_=elrow[:, :])
            val_s = cpool.tile([P, nt], BF16, tag="val")
            nc.scalar.dma_start(out=val_s[:], in_=eval_[:, :])
            idx_s = cpool.tile([P, nt * 8], I16, tag="idx")
            nc.sync.dma_start(out=idx_s[:], in_=eidx[:, :])
            x0t_s = cpool.tile([P, nblk * P], BF16, tag="x0t")
            nc.sync.dma_start(out=x0t_s[:], in_=x0t[:, :])
            x1t_s = cpool.tile([P, nblk * P], BF16, tag="x1t")

            qn = [0]

            def spmm(src_dram, layout_b, out_cb, after_sb_cb=None, gpool=None):
                for sb in range(es.nsb):
                    b0 = sb * SB
                    bh = min(SB, nblk - b0)
                    psums = {
                        b0 + bi: apool.tile(
                            [P, P], F32, tag="acc", name=f"acc{b0 + bi}"
                        )
                        for bi in range(bh)
                    }
                    for (r0, rntr, ch) in es.runs[sb]:
                      # one mask build per run (depends only on resident lrow)
                      mask = mpool.tile([P, GW * P], BF16, tag="m")
                      nc.vector.scalar_tensor_tensor(
                          out=mask[:, : rntr * P].rearrange(
                              "p (t j) -> p t j", j=P
                          ),
                          in0=iota_s[:, :]
                          .rearrange("p (o j) -> p o j", o=1)
                          .broadcast_to([P, rntr, P]),
                          scalar=0.0,
                          in1=lrow_s[:, r0 : r0 + rntr]
                          .rearrange("p (t o) -> p t o", o=1)
                          .broadcast_to([P, rntr, P]),
                          op0=AluOpType.bypass,
                          op1=AluOpType.is_equal,
                      )
                      for s0 in range(0, rntr, RT):
                        t0 = r0 + s0
                        ntr = min(RT, rntr - s0)
                        g = gpool.tile([P, RT * P], BF16, tag="g")
                        nidx = ntr * P
                        nc.gpsimd.dma_gather(
                            out_ap=g[:, :nidx].rearrange(
                                "p (t e) -> p t e", e=P
                            ),
                            in_ap=src_dram[
                                es.chunk_bounds[ch][0] : es.chunk_bounds[ch][1],
                                :,
                            ],
                            idxs_ap=idx_s[:, t0 * 8 : (t0 + ntr) * 8],
                            num_idxs=nidx,
                            num_idxs_reg=nidx,
                            elem_size=P,
                            single_packet=False,
                            queue_num=qn[0] % NQ,
                        )
                        qn[0] += 1
                        gs = mvpool.tile([P, RT * P], BF16, tag="mv")
                        nc.vector.scalar_tensor_tensor(
                            out=gs[:, :nidx].rearrange(
                                "p (t j) -> p t j", j=P
                            ),
                            in0=g[:, :nidx].rearrange("p (t j) -> p t j", j=P),
                            scalar=0.0,
                            in1=val_s[:, t0 : t0 + ntr]
                            .rearrange("p (t o) -> p t o", o=1)
                            .broadcast_to([P, ntr, P]),
                            op0=AluOpType.bypass,
                            op1=AluOpType.mult,
                        )
                        for tt in range(ntr):
                            t = t0 + tt
                            b = int(es.tile_block[t])
                            gt = gs[:, tt * P : (tt + 1) * P]
                            mm = mask[:, (s0 + tt) * P : (s0 + tt + 1) * P]
                            start = bool(es.tile_start[t])
                            stop = bool(es.tile_stop[t]) and not layout_b
                            if layout_b:
                                nc.tensor.matmul(
                                    out=psums[b][:], lhsT=gt, rhs=mm,
                                    start=start, stop=stop,
                                )
                            else:
                                nc.tensor.matmul(
                                    out=psums[b][:], lhsT=mm, rhs=gt,
                                    start=start, stop=stop,
                                )
                    for bi in range(bh):
                        out_cb(b0 + bi, psums[b0 + bi])
                    if after_sb_cb is not None:
                        after_sb_cb(sb)

            # ---------------- SpMM 1: x1 = L @ x0 (row-major out) --------
            def cb1(b, ps):
                xb = opool.tile([P, P], BF16, tag="x1st")
                nc.scalar.copy(out=xb[:], in_=ps[:])
                nc.sync.dma_start(
                    out=x1my[b * P : (b + 1) * P, :], in_=xb[:]
                )
                # x1^T block kept resident for the phase-2 channel mix
                pt = auxpool.tile([P, P], BF16, tag="aux", name="pt")
                nc.tensor.transpose(
                    out=pt[:], in_=xb[:], identity=ident_s[:]
                )
                nc.scalar.copy(
                    out=x1t_s[:, b * P : (b + 1) * P], in_=pt[:]
                )

            # piece-wise AllGather overlapping SpMM1's tail
            def gather_piece(sb):
                g = piece_after_sb.get(sb)
                if g is None:
                    return
                r0 = int(es.piece_start[g]) * P
                r1 = r0 + pb[g] * P
                d0 = int(es.piece_base[g])
                d1 = d0 + ncores * pb[g] * P
                nc.gpsimd.collective_compute(
                    "AllGather",
                    AluOpType.bypass,
                    replica_groups=[list(range(ncores))],
                    ins=[x1my[r0:r1, :].opt()],
                    outs=[x1full[d0:d1, :].opt()],
                )

            spmm(x0f, False, cb1, gather_piece, gpool=gpool0)

            # -------- SpMM 2 (transposed out) + fused channel mix --------
            def cb2(b, ps):
                # ps = (L x1)^T block; x2' = ps - 0.5 x0^T via fused DVE copy
                x0tb = x0t_s[:, b * P : (b + 1) * P]
                x2b = opool.tile([P, P], BF16, tag="x2b")
                nc.vector.scalar_tensor_tensor(
                    out=x2b[:],
                    in0=x0tb,
                    scalar=-0.5,
                    in1=ps[:],
                    op0=AluOpType.mult,
                    op1=AluOpType.add,
                )
                # channel mix: out = bias + sum_k Xk^T^T @ Wbd_k
                pm = auxpool.tile([P, P], F32, tag="aux", name="pm")
                nc.tensor.matmul(
                    out=pm[:], lhsT=ones_s[:], rhs=bias_s[:],
                    start=True, stop=False,
                )
                for k, xk in enumerate(
                    (x0tb, x1t_s[:, b * P : (b + 1) * P], x2b[:])
                ):
                    nc.tensor.matmul(
                        out=pm[:],
                        lhsT=xk,
                        rhs=wbd_s[:, k * P : (k + 1) * P],
                        start=False,
                        stop=(k == 2),
                    )
                ob = opool.tile([P, P], F32, tag="ob")
                nc.scalar.copy(out=ob[:], in_=pm[:])
                nc.scalar.dma_start(
                    out=outp[b * P : (b + 1) * P, :], in_=ob[:]
                )

            spmm(x1full, True, cb2, gpool=gpool0)

    nc.compile()
    return nc


# ---------------------------------------------------------------------------
# Host driver
# ---------------------------------------------------------------------------


def prepare(x, weight, bias, lap_vals, lap_rows, lap_cols, ncores=8):
    x = np.asarray(x, np.float32)
    weight = np.asarray(weight, np.float32)
    bias = np.asarray(bias, np.float32)
    lap_vals = np.asarray(lap_vals, np.float32)
    lap_rows = np.asarray(lap_rows)
    lap_cols = np.asarray(lap_cols)
    B, V, FIN = x.shape
    _, K, FOUT = weight.shape
    assert B == 2 and FIN == 64 and K == 3 and FOUT == 64

    es = EdgeStructure(V, ncores, lap_rows, lap_cols)

    x0 = np.concatenate([x[0], x[1]], axis=1).astype(NPBF16)  # [V, 128]
    x0f = es.x_to_flat(x0)

    wbd = np.zeros((3, P, P), np.float32)
    for k in range(3):
        wk = weight[:, k, :] * (2.0 if k == 2 else 1.0)  # x2' = x2/2
        wbd[k, :64, :64] = wk
        wbd[k, 64:, 64:] = wk
    wbd = wbd.astype(NPBF16)
    biasbd = np.concatenate([bias, bias]).reshape(1, P).astype(NPBF16)
    ident = np.eye(P, dtype=np.float32).astype(NPBF16)
    ones1 = np.ones((1, P), NPBF16)
    iota = np.broadcast_to(
        np.arange(P, dtype=np.float32)[None, :], (P, P)
    ).astype(NPBF16)

    in_maps = []
    for c in range(ncores):
        idx_w, lrow_c, val_c = es.per_core_arrays(c, lap_vals)
        x0sh = es.shard_permuted(x0, c)
        x0t_c = np.ascontiguousarray(
            x0sh.reshape(es.nblk, P, P).transpose(2, 0, 1).reshape(P, -1)
        )
        in_maps.append(
            {
                "x0f": x0f,
                "x0t": x0t_c,
                "wbd": wbd,
                "biasbd": biasbd,
                "ident": ident,
                "ones1": ones1,
                "iota": iota,
                "eidx": idx_w,
                "elrow": lrow_c,
                "eval": val_c,
            }
        )

    nc = build_program(es)

    def assemble(results):
        out = np.empty((B, V, FOUT), np.float32)
        for c in range(ncores):
            o = np.asarray(results[c]["outp"]).reshape(es.vpad, P)
            m = es.perms[c] >= 0
            orig = es.perms[c][m]
            out[0, c * es.vsh + orig, :] = o[m, :64]
            out[1, c * es.vsh + orig, :] = o[m, 64:]
        return out

    return nc, in_maps, assemble, es


def kernel(x, weight, bias, lap_vals, lap_rows, lap_cols):
    nc, in_maps, assemble, es = prepare(
        x, weight, bias, lap_vals, lap_rows, lap_cols
    )
    res = bass_utils.run_bass_kernel_spmd(
        nc, in_maps, core_ids=list(range(es.ncores))
    )
    return assemble(res.results)


# revision 39
# speedup vs baseline: 2.3896x; 2.3896x over previous
"""ChebConv (K=3) forward as a distributed Bass/Tile kernel on 8 trn2 NeuronCores.

Sharding: vertices V are sharded across the 8 cores (rows of L and x).
  x0 = [x[0] | x[1]]            # [V, 128], feature col = b*64 + fin
  x1 = L @ x0                   # SpMM (COO, edge-parallel)
  x2' = L @ x1 - 0.5 x0         # = x2/2; the 2x is folded into W_k2
  out[b,v,:] = bias + sum_k xk[v, b*64:(b+1)*64] @ Wk'

Each core owns a row shard (V/8 rows padded to 98 blocks of 128). SpMM per
core and per 128-edge tile (bf16 data, PSUM f32 accumulate):
  - gpsimd.dma_gather fetches 256B source feature rows from a flat-indexed
    replicated table (int16 indices, 4 chunks), round-robin over 4 SWDGE
    queues.
  - The selector mask[e,j] = (lrow[e]==j) is built ON-CHIP by the (otherwise
    idle) Vector engine from a streamed lrow row (one broadcast-compare per
    run); the edge value is folded into the gathered rows (gs = val * g,
    second DVE op). No selector matrix is ever streamed from HBM.
  - PE matmul mask^T @ gs (spmm1) or gs^T @ mask (spmm2) performs the scaled
    segmented sum into a per-block PSUM accumulator.
Edge metadata (idx / lrow / val) is loaded once into SBUF and shared by both
SpMMs. x0^T blocks are bulk-loaded resident; x1^T blocks are produced by a PE
transpose in phase 1 and kept resident, so phase 2 reloads nothing per-block.

The vertex flat index space is PIECE-major: blocks [0:28), [28:56), [56:84),
[84:98) of every core form 4 global pieces; the x1 AllGather is split into 4
piece collectives so the first 3 overlap with the tail of SpMM1. Gather
chunks coincide with pieces (each <= 32768 rows, int16-addressable).

The final channel mix is fused into SpMM2's block epilogue using
block-diagonal weights (both batches in one matmul chain) plus a rank-1 bias
matmul; -0.5 x0 enters via a DVE (x0^T * -0.5 + psum) fused copy.
"""

import sys

sys.path.insert(0, "/opt/trn_rl_repo")

import numpy as np
import ml_dtypes

import concourse.bass as bass
import concourse.bacc as bacc
import concourse.mybir as mybir
import concourse.tile as tile
from concourse import bass_utils
from concourse.alu_op_type import AluOpType

P = 128
F32 = mybir.dt.float32
BF16 = mybir.dt.bfloat16
I16 = mybir.dt.int16
NPBF16 = ml_dtypes.bfloat16
NQ = 4  # SWDGE queues (parallel Q7 descriptor generation)
SB = 7  # blocks per super-block (PSUM ring = SB accumulators)
RT = 14  # tiles per gather sub-run (small gathers overlap desc-gen better)
PIECE_BLOCKS = (28, 28, 28, 14)  # 4 pieces x (blocks/core); sbs of 7 align


def _cdiv(a, b):
    return -(-a // b)


# ---------------------------------------------------------------------------
# Host-side: uniform (cross-core) edge structure + per-core content arrays
# ---------------------------------------------------------------------------


def _pack_piece(dp, nb, caps, rounds=4, max_iter=60000):
    """Assign len(dp) rows (4-dim chunk-degree vectors) to nb blocks of 128
    rows each, fitting per-(block,chunk) counts under caps via swap search."""
    n = len(dp)
    order = np.argsort(-dp.sum(1), kind="stable")
    seq = np.empty(n, np.int64)
    blkseq = []
    fwd = True
    while len(blkseq) < n:
        blkseq.extend(range(nb) if fwd else range(nb - 1, -1, -1))
        fwd = not fwd
    seq[order] = np.array(blkseq[:n])
    assign = seq
    cnt = np.zeros((nb, 4), np.int64)
    np.add.at(cnt, (assign,), dp)
    it = 0
    for _ in range(rounds):
        blocked = np.zeros((nb, 4), bool)
        progress = False
        while it < max_iter:
            it += 1
            over = np.where(blocked, 0, cnt - caps)
            if over.max() <= 0:
                break
            b, g = np.unravel_index(np.argmax(over), over.shape)
            rows_b = np.nonzero(assign == b)[0]
            cand_i = rows_b[np.argsort(-dp[rows_b, g], kind="stable")[:4]]
            swapped = False
            for i in cand_i:
                di = dp[i]
                if di[g] == 0:
                    break
                f1 = np.all(cnt[assign] + di - dp <= caps[assign], axis=1)
                f1 &= assign != b
                after_b = cnt[b][None] - di[None] + dp
                f2 = np.all(
                    after_b <= np.maximum(caps[b][None], cnt[b][None]), axis=1
                )
                gain = di[g] - dp[:, g]
                ok = f1 & f2 & (gain > 0)
                if ok.any():
                    jj = np.nonzero(ok)[0]
                    j = jj[np.argmax(gain[jj])]
                    bj = assign[j]
                    assign[i], assign[j] = bj, b
                    cnt[b] += dp[j] - di
                    cnt[bj] += di - dp[j]
                    blocked[b, :] = False
                    blocked[bj, :] = False
                    swapped = True
                    progress = True
                    break
            if not swapped:
                blocked[b, g] = True
        if (cnt - caps).max() <= 0 or not progress:
            break
    return assign


class EdgeStructure:
    def __init__(self, V, ncores, rows, cols):
        assert V % ncores == 0
        self.V, self.ncores = V, ncores
        self.vsh = V // ncores
        self.nblk = _cdiv(self.vsh, P)
        self.vpad = self.nblk * P
        self.vtot = self.vpad * ncores
        assert sum(PIECE_BLOCKS) == self.nblk and self.nblk % SB == 0

        # piece-major flat layout: piece g holds ncores * PIECE_BLOCKS[g]
        # consecutive blocks (core-major within the piece)
        pb = np.asarray(PIECE_BLOCKS, np.int64)
        self.piece_start = np.concatenate(([0], np.cumsum(pb)))[:-1]
        self.piece_base = np.concatenate(([0], np.cumsum(pb * ncores * P)))[:-1]
        self.nchunks = len(pb)
        self.chunk_bounds = [
            (int(self.piece_base[g]), int(self.piece_base[g] + ncores * pb[g] * P))
            for g in range(self.nchunks)
        ]
        assert all(b - a <= 32768 for a, b in self.chunk_bounds)
        self.piece_of_block = np.searchsorted(
            self.piece_start, np.arange(self.nblk), side="right"
        ) - 1

        rows = np.asarray(rows, np.int64)
        cols = np.asarray(cols, np.int64)
        r_core = rows // self.vsh
        r_loc = rows - r_core * self.vsh
        c_of = cols // self.vsh
        l_of = cols - c_of * self.vsh

        # ---- row->block packing (within pieces) to minimize tile padding --
        colchunk0 = self.piece_of_block[l_of // P]
        d = np.zeros((ncores, self.vsh, 4), np.int32)
        np.add.at(d, (r_core, r_loc, colchunk0), 1)
        caps_all = np.zeros((self.nblk, 4), np.int64)
        for p in range(self.nchunks):
            b0, b1 = int(self.piece_start[p]), int(self.piece_start[p] + pb[p])
            nb = b1 - b0
            r0, r1 = b0 * P, min(b1 * P, self.vsh)
            tot = d[:, r0:r1].sum(axis=1)
            for g in range(self.nchunks):
                npg = -(-int(tot[:, g].max()) // P) + 1
                base, extra = npg // nb, npg % nb
                caps_all[b0:b1, g] = base * P
                caps_all[b0 : b0 + extra, g] += P
        self.perms = np.full((ncores, self.vpad), -1, np.int64)
        for c in range(ncores):
            for p in range(self.nchunks):
                b0 = int(self.piece_start[p])
                b1 = b0 + int(pb[p])
                nb = b1 - b0
                r0, r1 = b0 * P, min(b1 * P, self.vsh)
                n_real = r1 - r0
                n_tot = nb * P
                dp = np.zeros((n_tot, 4), np.int64)
                dp[:n_real] = d[c, r0:r1]
                assign = _pack_piece(dp, nb, caps_all[b0:b1])
                used = np.zeros(nb, np.int64)
                for idx in range(n_tot):
                    b = assign[idx]
                    self.perms[c, (b0 + b) * P + used[b]] = (
                        (r0 + idx) if idx < n_real else -1
                    )
                    used[b] += 1
        inv = np.full((ncores, self.vsh), -1, np.int64)
        for c in range(ncores):
            m = self.perms[c] >= 0
            inv[c, self.perms[c][m]] = np.nonzero(m)[0]

        # new (post-permutation) local positions for rows and columns
        r_loc = inv[r_core, r_loc]
        nl_col = inv[c_of, l_of]
        blk_col = nl_col // P
        g_col = self.piece_of_block[blk_col]
        flat = (
            self.piece_base[g_col]
            + c_of * pb[g_col] * P
            + (blk_col - self.piece_start[g_col]) * P
            + (nl_col - blk_col * P)
        )
        chunk = g_col
        blk = r_loc // P

        nchunks = self.nchunks
        # slot order: for sb: for chunk: for block in sb
        sb_arr = blk // SB
        bi_arr = blk % SB
        bh_arr = np.minimum(SB, self.nblk - sb_arr * SB)
        sid = sb_arr * SB * nchunks + chunk * bh_arr + bi_arr

        self.nsb = _cdiv(self.nblk, SB)
        order = []
        for sb in range(self.nsb):
            b0 = sb * SB
            bh = min(SB, self.nblk - b0)
            for ch in range(nchunks):
                for bi in range(bh):
                    order.append((b0 + bi, ch))
        self.nslots = len(order)
        self.slot_block = np.array([b for b, _ in order], np.int64)
        self.slot_chunk = np.array([c for _, c in order], np.int64)

        counts = np.zeros((ncores, self.nslots), np.int64)
        np.add.at(counts, (r_core, sid), 1)
        T = _cdiv(np.max(counts, axis=0), P)

        # every block needs >=1 tile so its PSUM accumulator gets written
        blk_tiles = np.zeros(self.nblk, np.int64)
        np.add.at(blk_tiles, self.slot_block, T)
        for b in np.nonzero(blk_tiles == 0)[0]:
            sb, bi = b // SB, b % SB
            bh = min(SB, self.nblk - sb * SB)
            T[sb * SB * nchunks + 0 * bh + bi] = 1

        self.T = T
        self.slot_tile_base = np.concatenate(([0], np.cumsum(T)))[:-1]
        self.ntiles = int(np.sum(T))
        self.sid_of_edge = sid
        self.flat_of_edge = flat
        self.chunk_of_edge = chunk
        self.r_core_of_edge = r_core
        self.lrow_of_edge = (r_loc % P).astype(np.int64)

        # (sb, chunk) -> contiguous tile run
        self.runs = []  # per sb: list of (tile_start, ntiles, chunk)
        s = 0
        for sb in range(self.nsb):
            b0 = sb * SB
            bh = min(SB, self.nblk - b0)
            sb_runs = []
            for ch in range(nchunks):
                t0 = int(self.slot_tile_base[s])
                ntr = int(np.sum(T[s : s + bh]))
                if ntr > 0:
                    sb_runs.append((t0, ntr, ch))
                s += bh
            self.runs.append(sb_runs)
        self.max_run_tiles = max(
            nt for sb_runs in self.runs for _, nt, _ in sb_runs
        )

        tile_block = np.empty(self.ntiles, np.int64)
        for s in range(self.nslots):
            t0, ntr = self.slot_tile_base[s], T[s]
            tile_block[t0 : t0 + ntr] = self.slot_block[s]
        self.tile_block = tile_block
        self.tile_start = np.zeros(self.ntiles, bool)
        self.tile_stop = np.zeros(self.ntiles, bool)
        first, last = {}, {}
        for t in range(self.ntiles):
            b = int(tile_block[t])
            if b not in first:
                first[b] = t
            last[b] = t
        for t in first.values():
            self.tile_start[t] = True
        for t in last.values():
            self.tile_stop[t] = True

    def per_core_arrays(self, core, vals):
        """int16 gather indices (wrapped+replicated) and bf16 lrow/val rows."""
        sel = np.nonzero(self.r_core_of_edge == core)[0]
        sid = self.sid_of_edge[sel]
        o = np.argsort(sid, kind="stable")
        sel, sid = sel[o], sid[o]
        start = np.searchsorted(sid, np.arange(self.nslots))
        rank = np.arange(len(sid)) - start[sid]
        pos = self.slot_tile_base[sid] * P + rank
        n = self.ntiles * P
        idx = np.zeros(n, np.int16)
        idx[pos] = (
            self.flat_of_edge[sel] - self.piece_base[self.slot_chunk[sid]]
        ).astype(np.int16)
        idx_w = np.tile(np.ascontiguousarray(idx.reshape(-1, 16).T), (8, 1))
        lrow = np.zeros((P, self.ntiles), np.float32)
        val = np.zeros((P, self.ntiles), np.float32)
        lrow[pos % P, pos // P] = self.lrow_of_edge[sel]
        val[pos % P, pos // P] = vals[sel]
        return idx_w, lrow.astype(NPBF16), val.astype(NPBF16)

    def shard_permuted(self, x0_bf16, c):
        """Core c's [vpad, 128] shard in post-packing row order."""
        xs = np.zeros((self.vpad, P), NPBF16)
        m = self.perms[c] >= 0
        xs[m] = x0_bf16[c * self.vsh + self.perms[c][m]]
        return xs

    def x_to_flat(self, x0_bf16):
        """Scatter [V, 128] rows into the piece-major padded flat table."""
        xf = np.zeros((self.vtot, P), NPBF16)
        pb = np.asarray(PIECE_BLOCKS, np.int64)
        for c in range(self.ncores):
            xs = self.shard_permuted(x0_bf16, c)
            for g in range(self.nchunks):
                r0 = self.piece_start[g] * P
                r1 = r0 + pb[g] * P
                dst = self.piece_base[g] + c * pb[g] * P
                xf[dst : dst + (r1 - r0)] = xs[r0:r1]
        return xf


# ---------------------------------------------------------------------------
# Bass program (SPMD: one program, per-core data via in_maps)
# ---------------------------------------------------------------------------


def build_program(es: EdgeStructure):
    nblk, vpad, vtot, ncores = es.nblk, es.vpad, es.vtot, es.ncores
    nt, GW = es.ntiles, es.max_run_tiles

    nc = bacc.Bacc(
        "TRN2",
        target_bir_lowering=False,
        debug=False,
        num_devices=ncores,
        num_swdge_queues=NQ,
    )

    x0f = nc.dram_tensor("x0f", [vtot, P], BF16, kind="ExternalInput")
    x0t = nc.dram_tensor("x0t", [P, nblk * P], BF16, kind="ExternalInput")
    wbd = nc.dram_tensor("wbd", [3, P, P], BF16, kind="ExternalInput")
    biasbd = nc.dram_tensor("biasbd", [1, P], BF16, kind="ExternalInput")
    ident_d = nc.dram_tensor("ident", [P, P], BF16, kind="ExternalInput")
    ones_d = nc.dram_tensor("ones1", [1, P], BF16, kind="ExternalInput")
    iota_d = nc.dram_tensor("iota", [P, P], BF16, kind="ExternalInput")
    eidx = nc.dram_tensor("eidx", [P, nt * 8], I16, kind="ExternalInput")
    elrow = nc.dram_tensor("elrow", [P, nt], BF16, kind="ExternalInput")
    eval_ = nc.dram_tensor("eval", [P, nt], BF16, kind="ExternalInput")
    outp = nc.dram_tensor("outp", [vpad, P], F32, kind="ExternalOutput")

    x1my = nc.dram_tensor("x1my", [vpad, P], BF16)
    x1full = nc.dram_tensor("x1full", [vtot, P], BF16, addr_space="Shared")

    pb = PIECE_BLOCKS
    nsb_total = sum(pb) // SB
    piece_after_sb = {}  # sb index -> piece idx launched at its end
    acc = 0
    for g, n in enumerate(pb):
        acc += n
        # launch one sb late (except the last piece): by then the piece's
        # x1my stores have drained, so the collective's sem-wait doesn't
        # head-of-line-block gather desc-gen on the gpsimd queue
        sb_done = acc // SB - 1
        piece_after_sb[min(sb_done + 1, nsb_total - 1)] = g

    with tile.TileContext(nc) as tc:
        with (
            tc.tile_pool(name="const", bufs=1) as cpool,
            tc.tile_pool(name="gslab", bufs=10) as gpool0,
            tc.tile_pool(name="mvslab", bufs=8) as mvpool,
            tc.tile_pool(name="mslab", bufs=8) as mpool,
            tc.tile_pool(name="ostage", bufs=4) as opool,
            tc.tile_pool(name="acc", bufs=SB, space="PSUM") as apool,
            tc.tile_pool(name="paux", bufs=1, space="PSUM") as auxpool,
        ):
            # ---- resident data (loaded once, shared by both SpMMs) ----
            ident_s = cpool.tile([P, P], BF16, tag="ident")
            nc.scalar.dma_start(out=ident_s[:], in_=ident_d[:, :])
            ones_s = cpool.tile([1, P], BF16, tag="ones")
            nc.scalar.dma_start(out=ones_s[:], in_=ones_d[:, :])
            bias_s = cpool.tile([1, P], BF16, tag="bias")
            nc.scalar.dma_start(out=bias_s[:], in_=biasbd[:, :])
            iota_s = cpool.tile([P, P], BF16, tag="iota")
            nc.scalar.dma_start(out=iota_s[:], in_=iota_d[:, :])
            wbd_s = cpool.tile([P, 3 * P], BF16, tag="wbd")
            for k in range(3):
                nc.scalar.dma_start(
                    out=wbd_s[:, k * P : (k + 1) * P], in_=wbd[k, :, :]
                )
            lrow_s = cpool.tile([P, nt], BF16, tag="lrow")
            nc.scalar.dma_start(out=lrow_s[:], in_=elrow[:, :])
            val_s = cpool.tile([P, nt], BF16, tag="val")
            nc.scalar.dma_start(out=val_s[:], in_=eval_[:, :])
            idx_s = cpool.tile([P, nt * 8], I16, tag="idx")
            nc.sync.dma_start(out=idx_s[:], in_=eidx[:, :])
            x0t_s = cpool.tile([P, nblk * P], BF16, tag="x0t")
            nc.sync.dma_start(out=x0t_s[:], in_=x0t[:, :])
            x1t_s = cpool.tile([P, nblk * P], BF16, tag="x1t")

            qn = [0]

            def spmm(src_dram, layout_b, out_cb, after_sb_cb=None, gpool=None):
                for sb in range(es.nsb):
                    b0 = sb * SB
                    bh = min(SB, nblk - b0)
                    psums = {
                        b0 + bi: apool.tile(
                            [P, P], F32, tag="acc", name=f"acc{b0 + bi}"
                        )
                        for bi in range(bh)
                    }
                    for (r0, rntr, ch) in es.runs[sb]:
                      for s0 in range(0, rntr, RT):
                        t0 = r0 + s0
                        ntr = min(RT, rntr - s0)
                        g = gpool.tile([P, RT * P], BF16, tag="g")
                        nidx = ntr * P
                        nc.gpsimd.dma_gather(
                            out_ap=g[:, :nidx].rearrange(
                                "p (t e) -> p t e", e=P
                            ),
                            in_ap=src_dram[
                                es.chunk_bounds[ch][0] : es.chunk_bounds[ch][1],
                                :,
                            ],
                            idxs_ap=idx_s[:, t0 * 8 : (t0 + ntr) * 8],
                            num_idxs=nidx,
                            num_idxs_reg=nidx,
                            elem_size=P,
                            single_packet=False,
                            queue_num=qn[0] % NQ,
                        )
                        qn[0] += 1
                        # on-chip selector mask + value-scaled gather rows
                        mask = mpool.tile([P, RT * P], BF16, tag="m")
                        nc.vector.scalar_tensor_tensor(
                            out=mask[:, :nidx].rearrange(
                                "p (t j) -> p t j", j=P
                            ),
                            in0=iota_s[:, :]
                            .rearrange("p (o j) -> p o j", o=1)
                            .broadcast_to([P, ntr, P]),
                            scalar=0.0,
                            in1=lrow_s[:, t0 : t0 + ntr]
                            .rearrange("p (t o) -> p t o", o=1)
                            .broadcast_to([P, ntr, P]),
                            op0=AluOpType.bypass,
                            op1=AluOpType.is_equal,
                        )
                        gs = mvpool.tile([P, RT * P], BF16, tag="mv")
                        nc.vector.scalar_tensor_tensor(
                            out=gs[:, :nidx].rearrange(
                                "p (t j) -> p t j", j=P
                            ),
                            in0=g[:, :nidx].rearrange("p (t j) -> p t j", j=P),
                            scalar=0.0,
                            in1=val_s[:, t0 : t0 + ntr]
                            .rearrange("p (t o) -> p t o", o=1)
                            .broadcast_to([P, ntr, P]),
                            op0=AluOpType.bypass,
                            op1=AluOpType.mult,
                        )
                        for tt in range(ntr):
                            t = t0 + tt
                            b = int(es.tile_block[t])
                            gt = gs[:, tt * P : (tt + 1) * P]
                            mm = mask[:, tt * P : (tt + 1) * P]
                            start = bool(es.tile_start[t])
                            stop = bool(es.tile_stop[t]) and not layout_b
                            if layout_b:
                                nc.tensor.matmul(
                                    out=psums[b][:], lhsT=gt, rhs=mm,
                                    start=start, stop=stop,
                                )
                            else:
                                nc.tensor.matmul(
                                    out=psums[b][:], lhsT=mm, rhs=gt,
                                    start=start, stop=stop,
                                )
                    for bi in range(bh):
                        out_cb(b0 + bi, psums[b0 + bi])
                    if after_sb_cb is not None:
                        after_sb_cb(sb)

            # ---------------- SpMM 1: x1 = L @ x0 (row-major out) --------
            def cb1(b, ps):
                xb = opool.tile([P, P], BF16, tag="x1st")
                nc.scalar.copy(out=xb[:], in_=ps[:])
                nc.sync.dma_start(
                    out=x1my[b * P : (b + 1) * P, :], in_=xb[:]
                )
                # x1^T block kept resident for the phase-2 channel mix
                pt = auxpool.tile([P, P], BF16, tag="aux", name="pt")
                nc.tensor.transpose(
                    out=pt[:], in_=xb[:], identity=ident_s[:]
                )
                nc.scalar.copy(
                    out=x1t_s[:, b * P : (b + 1) * P], in_=pt[:]
                )

            # piece-wise AllGather overlapping SpMM1's tail
            def gather_piece(sb):
                g = piece_after_sb.get(sb)
                if g is None:
                    return
                r0 = int(es.piece_start[g]) * P
                r1 = r0 + pb[g] * P
                d0 = int(es.piece_base[g])
                d1 = d0 + ncores * pb[g] * P
                nc.gpsimd.collective_compute(
                    "AllGather",
                    AluOpType.bypass,
                    replica_groups=[list(range(ncores))],
                    ins=[x1my[r0:r1, :].opt()],
                    outs=[x1full[d0:d1, :].opt()],
                )

            spmm(x0f, False, cb1, gather_piece, gpool=gpool0)

            # -------- SpMM 2 (transposed out) + fused channel mix --------
            def cb2(b, ps):
                # ps = (L x1)^T block; x2' = ps - 0.5 x0^T via fused DVE copy
                x0tb = x0t_s[:, b * P : (b + 1) * P]
                x2b = opool.tile([P, P], BF16, tag="x2b")
                nc.vector.scalar_tensor_tensor(
                    out=x2b[:],
                    in0=x0tb,
                    scalar=-0.5,
                    in1=ps[:],
                    op0=AluOpType.mult,
                    op1=AluOpType.add,
                )
                # channel mix: out = bias + sum_k Xk^T^T @ Wbd_k
                pm = auxpool.tile([P, P], F32, tag="aux", name="pm")
                nc.tensor.matmul(
                    out=pm[:], lhsT=ones_s[:], rhs=bias_s[:],
                    start=True, stop=False,
                )
                for k, xk in enumerate(
                    (x0tb, x1t_s[:, b * P : (b + 1) * P], x2b[:])
                ):
                    nc.tensor.matmul(
                        out=pm[:],
                        lhsT=xk,
                        rhs=wbd_s[:, k * P : (k + 1) * P],
                        start=False,
                        stop=(k == 2),
                    )
                ob = opool.tile([P, P], F32, tag="ob")
                nc.scalar.copy(out=ob[:], in_=pm[:])
                nc.scalar.dma_start(
                    out=outp[b * P : (b + 1) * P, :], in_=ob[:]
                )

            spmm(x1full, True, cb2, gpool=gpool0)

    nc.compile()
    return nc


# ---------------------------------------------------------------------------
# Host driver
# ---------------------------------------------------------------------------


def prepare(x, weight, bias, lap_vals, lap_rows, lap_cols, ncores=8):
    x = np.asarray(x, np.float32)
    weight = np.asarray(weight, np.float32)
    bias = np.asarray(bias, np.float32)
    lap_vals = np.asarray(lap_vals, np.float32)
    lap_rows = np.asarray(lap_rows)
    lap_cols = np.asarray(lap_cols)
    B, V, FIN = x.shape
    _, K, FOUT = weight.shape
    assert B == 2 and FIN == 64 and K == 3 and FOUT == 64

    es = EdgeStructure(V, ncores, lap_rows, lap_cols)

    x0 = np.concatenate([x[0], x[1]], axis=1).astype(NPBF16)  # [V, 128]
    x0f = es.x_to_flat(x0)

    wbd = np.zeros((3, P, P), np.float32)
    for k in range(3):
        wk = weight[:, k, :] * (2.0 if k == 2 else 1.0)  # x2' = x2/2
        wbd[k, :64, :64] = wk
        wbd[k, 64:, 64:] = wk
    wbd = wbd.astype(NPBF16)
    biasbd = np.concatenate([bias, bias]).reshape(1, P).astype(NPBF16)
    ident = np.eye(P, dtype=np.float32).astype(NPBF16)
    ones1 = np.ones((1, P), NPBF16)
    iota = np.broadcast_to(
        np.arange(P, dtype=np.float32)[None, :], (P, P)
    ).astype(NPBF16)

    in_maps = []
    for c in range(ncores):
        idx_w, lrow_c, val_c = es.per_core_arrays(c, lap_vals)
        x0sh = es.shard_permuted(x0, c)
        x0t_c = np.ascontiguousarray(
            x0sh.reshape(es.nblk, P, P).transpose(2, 0, 1).reshape(P, -1)
        )
        in_maps.append(
            {
                "x0f": x0f,
                "x0t": x0t_c,
                "wbd": wbd,
                "biasbd": biasbd,
                "ident": ident,
                "ones1": ones1,
                "iota": iota,
                "eidx": idx_w,
                "elrow": lrow_c,
                "eval": val_c,
            }
        )

    nc = build_program(es)

    def assemble(results):
        out = np.empty((B, V, FOUT), np.float32)
        for c in range(ncores):
            o = np.asarray(results[c]["outp"]).reshape(es.vpad, P)
            m = es.perms[c] >= 0
            orig = es.perms[c][m]
            out[0, c * es.vsh + orig, :] = o[m, :64]
            out[1, c * es.vsh + orig, :] = o[m, 64:]
        return out

    return nc, in_maps, assemble, es


def kernel(x, weight, bias, lap_vals, lap_rows, lap_cols):
    nc, in_maps, assemble, es = prepare(
        x, weight, bias, lap_vals, lap_rows, lap_cols
    )
    res = bass_utils.run_bass_kernel_spmd(
        nc, in_maps, core_ids=list(range(es.ncores))
    )
    return assemble(res.results)
